# revision 9
# baseline (speedup 1.0000x reference)
"""Trainium2 Bass kernel for nn_BetaVAEMark10Decoder.

Network (per sample): latent(4) -> Linear(256)+leaky -> reshape (1,8,32)
 -> convT(5,2)s(5,2) -> conv3x3 SAME +leaky   (5,16,16)
 -> convT(5,2)s(5,2) -> conv3x3 SAME +leaky   (25,32,8)
 -> convT(2,2)s(2,2) -> conv3x3 SAME +relu    (50,64,6)  -> NCHW out.

Each convT(k=s) + 3x3 pair composes into one exact linear map, block-banded
over rows: out row y reads 1-2 input rows through per-phase matrices.

Cost model facts this kernel is built around:
  * matmul cost = out_free_size x cycles_per_row (independent of K);
    f32r is 1 cyc/row only when N >= 256; fp16/bf16 are 1 cyc/row always.
  * DMA transfers serialize at ~360 GB/s -> write the output as fp16.
  * Activation cost = free_size x 0.83ns + ~0.4us fixed -> merge acts
    across PSUM banks.

Layers:
  L1-L3 run form 0 (feature-major, N=512 batch free) in f32r.
  L4 runs form 1 (batch-major out) in fp16 with support-split x-groups:
    out cols x in [0,31) only need input cols j0-15 (one 128-part chunk),
    x in [33,64) only need j16-31, and x in {31,32} get two tiny K=32
    matmuls (N=12) reading aligned partition slices with zero-padded
    weight rows.  This halves L4 PE time vs. the 2-pass K-chunk split.

Sharding: pure data parallel, batch 4096 -> 8 cores x 512.
"""

import sys

import numpy as np

sys.path.insert(0, "/opt/trn_rl_repo")

import concourse.bass as bass  # noqa: E402
import concourse.bacc as bacc  # noqa: E402
import concourse.mybir as mybir  # noqa: E402
from concourse import tile  # noqa: E402
from concourse.bass_utils import run_bass_kernel_spmd  # noqa: E402

N_CORES = 8
B = 4096
BL = B // N_CORES  # 512 per core
F32 = mybir.dt.float32
F32R = mybir.dt.float32r
F16 = mybir.dt.float16


# ---------------------------------------------------------------- host math
def _fused_matrices(Wup, Wc, sy, sx, Win, in_idx, out_idx, n_out_cols):
    """Compose convT(k=s=(sy,sx)) with 3x3 SAME conv into per-phase row
    matrices.  Returns {(p, delta): M} where out row y (p = y%sy, i = y//sy)
    accumulates  in_row[i+delta] @ M[(p, delta)]  over available deltas.
    x-edge clipping is baked into M; y-edge clipping == skipping absent rows.
    """
    Wup = np.asarray(Wup, np.float32)
    Wc = np.asarray(Wc, np.float32)
    Cin = Wup.shape[2]
    Wout = Win * sx
    mats = {}
    for p in range(sy):
        deltas = {0}
        if p == 0:
            deltas.add(-1)
        if p == sy - 1:
            deltas.add(1)
        for d in sorted(deltas):
            M = np.zeros((Win * Cin, n_out_cols), np.float32)
            y = sy + p  # representative interior row
            i_t = y // sy + d
            nz = False
            for dy in (-1, 0, 1):
                yp = y + dy
                if yp // sy != i_t:
                    continue
                py = yp % sy
                for x in range(Wout):
                    for dx in (-1, 0, 1):
                        xp = x + dx
                        if xp < 0 or xp >= Wout:
                            continue
                        j, qx = divmod(xp, sx)
                        # conv_transpose (transpose_kernel=False) applies the
                        # spatially mirrored kernel per phase
                        CC = Wup[sy - 1 - py, sx - 1 - qx] @ Wc[dy + 1, dx + 1]
                        M[np.ix_(in_idx(j), out_idx(x))] += CC
                        nz = True
            if nz:
                mats[(p, d)] = M
    return mats


def build_host_matrices(W_lin, W_up1, W_c1, W_up2, W_c2, W_up3, W_c3):
    # L2 input = h natural ordering: feat = c*8 + j   (c<32, j<8)
    r2 = _fused_matrices(
        W_up1, W_c1, 5, 2, 8,
        in_idx=lambda j: np.arange(32) * 8 + j,
        out_idx=lambda x: x * 16 + np.arange(16),
        n_out_cols=256,
    )
    # L3 input ordering: feat = j*16 + c ; output feat = x*8 + o
    r3 = _fused_matrices(
        W_up2, W_c2, 5, 2, 16,
        in_idx=lambda j: j * 16 + np.arange(16),
        out_idx=lambda x: x * 8 + np.arange(8),
        n_out_cols=256,
    )
    # L4 input ordering: feat = j*8 + c ; output col = o*64 + x  (x contig)
    r4 = _fused_matrices(
        W_up3, W_c3, 2, 2, 32,
        in_idx=lambda j: j * 8 + np.arange(8),
        out_idx=lambda x: x + 64 * np.arange(6),
        n_out_cols=384,
    )
    return np.asarray(W_lin, np.float32), r2, r3, r4


def _contribs(p, i, n_in_rows, mats, sy):
    out = []
    for d in (-1, 0, 1):
        if (p, d) in mats and 0 <= i + d < n_in_rows:
            out.append((i + d, mats[(p, d)]))
    return out


def numpy_forward(latent, W_lin, b_lin, r2, r3, r4):
    """Pure-numpy forward through the fused matrices (golden check)."""
    def leaky(x):
        return np.where(x > 0, x, 0.01 * x)

    h = leaky(latent.astype(np.float32) @ W_lin + b_lin)  # [B, 256]
    rows = h[:, None, :]  # [B, 1, 256]
    for (mats, sy, n_in) in ((r2, 5, 1), (r3, 5, 5)):
        nrows = n_in * sy
        out = np.zeros((h.shape[0], nrows, 256), np.float32)
        for y in range(nrows):
            i, p = divmod(y, sy)
            for (src, M) in _contribs(p, i, n_in, mats, sy):
                out[:, y] += rows[:, src] @ M
        rows = leaky(out)
    out = np.zeros((h.shape[0], 50, 384), np.float32)
    for y in range(50):
        i, p = divmod(y, 2)
        for (src, M) in _contribs(p, i, 25, r4, 2):
            out[:, y] += rows[:, src] @ M
    out = np.maximum(out, 0.0)
    # [B, 50, 6, 64] -> NCHW [B, 6, 50, 64]
    return out.reshape(-1, 50, 6, 64).transpose(0, 2, 1, 3)


# ---------------------------------------------------------------- bass build
_CACHED = {}

# L4 support-split column groups (out col = o*64 + x):
#   A0: x in [0, 31)  -> needs only j0-15  (input partitions   0:128)
#   A1: x in [33, 64) -> needs only j16-31 (input partitions 128:256)
#   mid: x in {31, 32} -> j15 (parts 96:128 of chunk0, rows zero-padded)
#                        + j16 (parts 0:32 of chunk1, rows zero-padded)
XA0 = list(range(0, 31))
XA1 = list(range(33, 64))
XMID = [31, 32]


def _mat_names(tag, mats):
    return {k: f"{tag}_{k[0]}_{'m' if k[1] < 0 else 'p'}{abs(k[1])}" for k in mats}


def build_nc(r2_keys, r3_keys, r4_keys):
    nc = bacc.Bacc('TRN2', target_bir_lowering=False, debug=False,
                   num_devices=N_CORES)

    lat = nc.declare_dram_parameter("latent_t", [4, BL], F32R, isOutput=False)
    w1 = nc.declare_dram_parameter("w1", [4, 256], F32R, isOutput=False)
    blin = nc.declare_dram_parameter("bl", [128, 2], F32, isOutput=False)
    r2n = _mat_names("r2", r2_keys)
    r3n = _mat_names("r3", r3_keys)
    r4n = _mat_names("r4", r4_keys)
    rd = {}
    for k, nm in list(r2n.items()) + list(r3n.items()):
        rd[nm] = nc.declare_dram_parameter(nm, [128, 2, 256], F16, isOutput=False)
    for k, nm in r4n.items():
        # support-split pieces of the [256, 384] fused L4 matrix
        rd[nm + "_a0"] = nc.declare_dram_parameter(
            nm + "_a0", [128, 6, len(XA0)], F16, isOutput=False)
        rd[nm + "_a1"] = nc.declare_dram_parameter(
            nm + "_a1", [128, 6, len(XA1)], F16, isOutput=False)
        rd[nm + "_m0"] = nc.declare_dram_parameter(
            nm + "_m0", [128, 6, 2], F16, isOutput=False)
        rd[nm + "_m1"] = nc.declare_dram_parameter(
            nm + "_m1", [128, 6, 2], F16, isOutput=False)
    # out stored (b, y, o, x) fp16; host transposes to NCHW + upcasts
    out = nc.declare_dram_parameter("out", [BL, 50, 6, 64], F16, isOutput=True)

    LR = mybir.ActivationFunctionType.Lrelu
    RELU = mybir.ActivationFunctionType.Relu

    with tile.TileContext(nc) as tc:
        with (
            tc.tile_pool(name="wpool", bufs=1) as wp,
            tc.tile_pool(name="acts", bufs=1) as ap,
            tc.tile_pool(name="ps", bufs=2, space=bass.MemorySpace.PSUM) as pp,
            tc.tile_pool(name="outp", bufs=3) as op,
        ):
            w1_t = wp.tile([4, 256], F32R, tag="w1")
            nc.sync.dma_start(out=w1_t[:], in_=w1[:])
            bl_t = wp.tile([128, 2], F32, tag="bl")
            nc.sync.dma_start(out=bl_t[:], in_=blin[:])
            lat_t = wp.tile([4, BL], F32R, tag="lat")
            nc.sync.dma_start(out=lat_t[:], in_=lat[:])

            def load(nm, shape, dt):
                t = wp.tile(shape, dt, tag=nm)
                nc.sync.dma_start(out=t[:], in_=rd[nm][:])
                return t

            r2_t = {k: load(nm, [128, 2, 256], F16) for k, nm in r2n.items()}
            r3_t = {k: load(nm, [128, 2, 256], F16) for k, nm in r3n.items()}
            r4_t = {}
            for k, nm in r4n.items():
                r4_t[k] = (
                    load(nm + "_a0", [128, 6, len(XA0)], F16),
                    load(nm + "_a1", [128, 6, len(XA1)], F16),
                    load(nm + "_m0", [128, 6, 2], F16),
                    load(nm + "_m1", [128, 6, 2], F16),
                )

            # ---- L1: h[256, B] = leaky(W_lin.T @ lat + b)  (2 banks, 2 acts
            # kept separate because bias differs per 128-chunk)
            x1 = ap.tile([128, 2, BL], F16, tag="x1")
            ps1 = pp.tile([128, 4, 8, 64], F32, tag="ps")
            for mc in range(2):
                nc.tensor.matmul(
                    ps1[:, mc, :, :], w1_t[:, bass.ts(mc, 128)], lat_t[:],
                    start=True, stop=True,
                )
                nc.scalar.activation(
                    x1[:, mc, :], ps1[:, mc, :, :], LR,
                    bias=bl_t[:, mc:mc + 1], alpha=0.01,
                )

            # ---- L2: 256 -> 1280 (5 rows x 256), input has 1 row.
            # x2 slot layout: slot 2*y + mc.  Acts merged per 2-row group.
            x2 = ap.tile([128, 10, BL], F16, tag="x2")
            for g0 in range(0, 5, 2):
                rows = [g0] if g0 == 4 else [g0, g0 + 1]
                ps = pp.tile([128, 4, 8, 64], F32, tag="ps")
                for r, y in enumerate(rows):
                    cs = _contribs(y, 0, 1, r2_t, 5)
                    for mc in range(2):
                        n, tot = 0, len(cs) * 2
                        for (src, mt) in cs:
                            for kc in range(2):
                                nc.tensor.matmul(
                                    ps[:, 2 * r + mc, :, :],
                                    mt[:, kc, bass.ts(mc, 128)],
                                    x1[:, kc, :],
                                    start=(n == 0), stop=(n == tot - 1),
                                )
                                n += 1
                nsl = 2 * len(rows)
                tmp = ap.tile([128, 4, 8, 64], F32, tag="tmp")
                nc.vector.tensor_scalar_mul(
                    tmp[:, 0:nsl, :, :], ps[:, 0:nsl, :, :], 0.01)
                nc.vector.scalar_tensor_tensor(
                    x2[:, 2 * g0:2 * g0 + nsl, :], ps[:, 0:nsl, :, :], 1.0,
                    tmp[:, 0:nsl, :, :],
                    op0=mybir.AluOpType.mult, op1=mybir.AluOpType.max,
                )

            # ---- L3: 1280 -> 6400 (25 rows x 256), fp16 out for L4.
            # x3 slot layout: slot 2*y + mc  (mc=0 -> j0-15, mc=1 -> j16-31).
            x3 = ap.tile([128, 50, BL], F16, tag="x3")
            for g0 in range(0, 25, 2):
                rows = [g0] if g0 == 24 else [g0, g0 + 1]
                ps = pp.tile([128, 4, 8, 64], F32, tag="ps")
                for r, y in enumerate(rows):
                    i, p = divmod(y, 5)
                    cs = _contribs(p, i, 5, r3_t, 5)
                    for mc in range(2):
                        n, tot = 0, len(cs) * 2
                        for (src, mt) in cs:
                            for kc in range(2):
                                nc.tensor.matmul(
                                    ps[:, 2 * r + mc, :, :],
                                    mt[:, kc, bass.ts(mc, 128)],
                                    x2[:, 2 * src + kc, :],
                                    start=(n == 0), stop=(n == tot - 1),
                                )
                                n += 1
                    nc.scalar.activation(
                        x3[:, 2 * y:2 * y + 2, :],
                        ps[:, 2 * r:2 * r + 2, :, :], LR, alpha=0.01,
                    )

            # ---- L4 (form 1, fp16): 6400 -> 19200, batch-major, relu, DMA.
            # Per 4-row y-group: one PSUM tile [128, 4, 8, 64] (4 banks,
            # o-dim padded 6->8 for bank alignment), support-split matmuls,
            # one merged relu act (alternating Scalar/Vector), one DMA.
            ygroups = [list(range(g, min(g + 4, 50))) for g in range(0, 50, 4)]
            actsel = 0
            for bb in range(BL // 128):
                for ys in ygroups:
                    pc = pp.tile([128, 4, 8, 64], F32, tag="ps")
                    for yy, y in enumerate(ys):
                        i, p = divmod(y, 2)
                        cs = _contribs(p, i, 25, r4_t, 2)
                        nmm, tot = 0, len(cs) * 4
                        for (src, (wa0, wa1, wm0, wm1)) in cs:
                            la = x3[:, 2 * src, bass.ts(bb, 128)]
                            lb = x3[:, 2 * src + 1, bass.ts(bb, 128)]
                            nc.tensor.matmul(
                                pc[:, yy, 0:6, 0:31], la, wa0[:],
                                start=(nmm == 0), stop=(nmm == tot - 4),
                            )
                            nc.tensor.matmul(
                                pc[:, yy, 0:6, 33:64], lb, wa1[:],
                                start=False, stop=(nmm == tot - 4),
                            )
                            nc.tensor.matmul(
                                pc[:, yy, 0:6, 31:33], la, wm0[:],
                                start=False, stop=False,
                            )
                            nc.tensor.matmul(
                                pc[:, yy, 0:6, 31:33], lb, wm1[:],
                                start=False, stop=(nmm == tot - 4),
                            )
                            nmm += 4
                    ob = op.tile([128, 4, 6, 64], F16, tag="ob")
                    nsl = len(ys)
                    for h0 in range(0, nsl, 2):
                        h1 = min(h0 + 2, nsl)
                        eng = (nc.scalar, nc.vector)[actsel % 2]
                        actsel += 1
                        if eng is nc.scalar:
                            nc.scalar.activation(
                                ob[:, h0:h1, :, :], pc[:, h0:h1, 0:6, :], RELU)
                        else:
                            eng.tensor_scalar_max(
                                ob[:, h0:h1, :, :], pc[:, h0:h1, 0:6, :], 0.0)
                    nc.sync.dma_start(
                        out=out[bass.ts(bb, 128), ys[0]:ys[0] + nsl, :, :],
                        in_=ob[:, 0:nsl, :, :],
                    )
    nc.compile()
    return nc


# ---------------------------------------------------------------- entry
def kernel(**inputs):
    latent = np.asarray(inputs["latent_vector"], np.float32)
    W_lin, r2, r3, r4 = build_host_matrices(
        inputs["W_lin"], inputs["W_up1"], inputs["W_c1"],
        inputs["W_up2"], inputs["W_c2"], inputs["W_up3"], inputs["W_c3"],
    )
    b_lin = np.asarray(inputs["b_lin"], np.float32)

    if "nc" not in _CACHED:
        _CACHED["nc"] = build_nc(r2, r3, r4)
    nc = _CACHED["nc"]

    def pack(m):  # [256, N] -> [128, 2, N]
        return np.ascontiguousarray(
            m.reshape(2, 128, m.shape[1]).transpose(1, 0, 2))

    base = {"w1": np.ascontiguousarray(W_lin),
            "bl": np.ascontiguousarray(b_lin.reshape(2, 128).T)}
    for tag, mats in (("r2", r2), ("r3", r3)):
        for k, nm in _mat_names(tag, mats).items():
            base[nm] = pack(mats[k]).astype(np.float16)
    # L4 support-split weight pieces; cols of the [256, 384] matrix are
    # o*64 + x.  a0: rows j0-15 for x in XA0; a1: rows j16-31 for x in XA1;
    # m0/m1: rows j12-15 / j16-19 (only j15/j16 nonzero) for x in XMID.
    for k, nm in _mat_names("r4", r4).items():
        M = r4[k]  # [256, 384]
        cview = M.reshape(256, 6, 64)
        base[nm + "_a0"] = np.ascontiguousarray(
            cview[0:128][:, :, XA0].astype(np.float16))
        base[nm + "_a1"] = np.ascontiguousarray(
            cview[128:256][:, :, XA1].astype(np.float16))
        base[nm + "_m0"] = np.ascontiguousarray(
            cview[0:128][:, :, XMID].astype(np.float16))
        base[nm + "_m1"] = np.ascontiguousarray(
            cview[128:256][:, :, XMID].astype(np.float16))

    in_maps = []
    for c in range(N_CORES):
        sh = latent[c * BL:(c + 1) * BL]
        in_maps.append({**base,
                        "latent_t": np.ascontiguousarray(sh.T)})

    _CACHED["maps"] = in_maps
    res = run_bass_kernel_spmd(nc, in_maps, list(range(N_CORES)))
    outs = [np.transpose(r["out"].astype(np.float32), (0, 2, 1, 3))
            for r in res.results]
    return np.ascontiguousarray(np.concatenate(outs, axis=0))


if __name__ == "__main__":
    rng = np.random.default_rng(0)
    fake = {
        "latent_vector": rng.standard_normal((B, 4)).astype(np.float32),
        "W_lin": rng.standard_normal((4, 256)).astype(np.float32) * 0.5,
        "b_lin": np.zeros(256, np.float32),
        "W_up1": rng.standard_normal((5, 2, 32, 32)).astype(np.float32) * 0.1,
        "W_c1": rng.standard_normal((3, 3, 32, 16)).astype(np.float32) * 0.1,
        "W_up2": rng.standard_normal((5, 2, 16, 16)).astype(np.float32) * 0.1,
        "W_c2": rng.standard_normal((3, 3, 16, 8)).astype(np.float32) * 0.1,
        "W_up3": rng.standard_normal((2, 2, 8, 8)).astype(np.float32) * 0.1,
        "W_c3": rng.standard_normal((3, 3, 8, 6)).astype(np.float32) * 0.1,
    }
    o = kernel(**fake)
    print("kernel out", o.shape, o.dtype)


# revision 10
# speedup vs baseline: 1.3184x; 1.3184x over previous
"""Trainium2 Bass kernel for nn_BetaVAEMark10Decoder.

Network (per sample): latent(4) -> Linear(256)+leaky -> reshape (1,8,32)
 -> convT(5,2)s(5,2) -> conv3x3 SAME +leaky   (5,16,16)
 -> convT(5,2)s(5,2) -> conv3x3 SAME +leaky   (25,32,8)
 -> convT(2,2)s(2,2) -> conv3x3 SAME +relu    (50,64,6)  -> NCHW out.

Each convT(k=s) + 3x3 pair composes into one exact linear map, block-banded
over rows: out row y reads 1-2 input rows through per-phase matrices.

Cost model facts this kernel is built around:
  * matmul cost = out_free_size x cycles_per_row (independent of K);
    f32r is 1 cyc/row only when N >= 256; fp16/bf16 are 1 cyc/row always.
  * DMA transfers serialize at ~360 GB/s -> write the output as fp16.
  * Activation cost = free_size x 0.83ns + ~0.4us fixed -> merge acts
    across PSUM banks.

Layers:
  L1-L3 run form 0 (feature-major, N=512 batch free) in f32r.
  L4 runs form 1 (batch-major out) in fp16 with support-split x-groups:
    out cols x in [0,31) only need input cols j0-15 (one 128-part chunk),
    x in [33,64) only need j16-31, and x in {31,32} get two tiny K=32
    matmuls (N=12) reading aligned partition slices with zero-padded
    weight rows.  This halves L4 PE time vs. the 2-pass K-chunk split.

Sharding: pure data parallel, batch 4096 -> 8 cores x 512.
"""

import sys

import numpy as np

sys.path.insert(0, "/opt/trn_rl_repo")

import concourse.bass as bass  # noqa: E402
import concourse.bacc as bacc  # noqa: E402
import concourse.mybir as mybir  # noqa: E402
from concourse import tile  # noqa: E402
from concourse.bass_utils import run_bass_kernel_spmd  # noqa: E402

N_CORES = 8
B = 4096
BL = B // N_CORES  # 512 per core
F32 = mybir.dt.float32
F32R = mybir.dt.float32r
F16 = mybir.dt.float16


# ---------------------------------------------------------------- host math
def _fused_matrices(Wup, Wc, sy, sx, Win, in_idx, out_idx, n_out_cols):
    """Compose convT(k=s=(sy,sx)) with 3x3 SAME conv into per-phase row
    matrices.  Returns {(p, delta): M} where out row y (p = y%sy, i = y//sy)
    accumulates  in_row[i+delta] @ M[(p, delta)]  over available deltas.
    x-edge clipping is baked into M; y-edge clipping == skipping absent rows.
    """
    Wup = np.asarray(Wup, np.float32)
    Wc = np.asarray(Wc, np.float32)
    Cin = Wup.shape[2]
    Wout = Win * sx
    mats = {}
    for p in range(sy):
        deltas = {0}
        if p == 0:
            deltas.add(-1)
        if p == sy - 1:
            deltas.add(1)
        for d in sorted(deltas):
            M = np.zeros((Win * Cin, n_out_cols), np.float32)
            y = sy + p  # representative interior row
            i_t = y // sy + d
            nz = False
            for dy in (-1, 0, 1):
                yp = y + dy
                if yp // sy != i_t:
                    continue
                py = yp % sy
                for x in range(Wout):
                    for dx in (-1, 0, 1):
                        xp = x + dx
                        if xp < 0 or xp >= Wout:
                            continue
                        j, qx = divmod(xp, sx)
                        # conv_transpose (transpose_kernel=False) applies the
                        # spatially mirrored kernel per phase
                        CC = Wup[sy - 1 - py, sx - 1 - qx] @ Wc[dy + 1, dx + 1]
                        M[np.ix_(in_idx(j), out_idx(x))] += CC
                        nz = True
            if nz:
                mats[(p, d)] = M
    return mats


def build_host_matrices(W_lin, W_up1, W_c1, W_up2, W_c2, W_up3, W_c3):
    # L2 input = h natural ordering: feat = c*8 + j   (c<32, j<8)
    r2 = _fused_matrices(
        W_up1, W_c1, 5, 2, 8,
        in_idx=lambda j: np.arange(32) * 8 + j,
        out_idx=lambda x: x * 16 + np.arange(16),
        n_out_cols=256,
    )
    # L3 input ordering: feat = j*16 + c ; output feat = x*8 + o
    r3 = _fused_matrices(
        W_up2, W_c2, 5, 2, 16,
        in_idx=lambda j: j * 16 + np.arange(16),
        out_idx=lambda x: x * 8 + np.arange(8),
        n_out_cols=256,
    )
    # L4 input ordering: feat = j*8 + c ; output col = o*64 + x  (x contig)
    r4 = _fused_matrices(
        W_up3, W_c3, 2, 2, 32,
        in_idx=lambda j: j * 8 + np.arange(8),
        out_idx=lambda x: x + 64 * np.arange(6),
        n_out_cols=384,
    )
    return np.asarray(W_lin, np.float32), r2, r3, r4


def _contribs(p, i, n_in_rows, mats, sy):
    out = []
    for d in (-1, 0, 1):
        if (p, d) in mats and 0 <= i + d < n_in_rows:
            out.append((i + d, mats[(p, d)]))
    return out


def numpy_forward(latent, W_lin, b_lin, r2, r3, r4):
    """Pure-numpy forward through the fused matrices (golden check)."""
    def leaky(x):
        return np.where(x > 0, x, 0.01 * x)

    h = leaky(latent.astype(np.float32) @ W_lin + b_lin)  # [B, 256]
    rows = h[:, None, :]  # [B, 1, 256]
    for (mats, sy, n_in) in ((r2, 5, 1), (r3, 5, 5)):
        nrows = n_in * sy
        out = np.zeros((h.shape[0], nrows, 256), np.float32)
        for y in range(nrows):
            i, p = divmod(y, sy)
            for (src, M) in _contribs(p, i, n_in, mats, sy):
                out[:, y] += rows[:, src] @ M
        rows = leaky(out)
    out = np.zeros((h.shape[0], 50, 384), np.float32)
    for y in range(50):
        i, p = divmod(y, 2)
        for (src, M) in _contribs(p, i, 25, r4, 2):
            out[:, y] += rows[:, src] @ M
    out = np.maximum(out, 0.0)
    # [B, 50, 6, 64] -> NCHW [B, 6, 50, 64]
    return out.reshape(-1, 50, 6, 64).transpose(0, 2, 1, 3)


# ---------------------------------------------------------------- bass build
_CACHED = {}

# L4 support-split column groups (out col = o*64 + x):
#   A0: x in [0, 31)  -> needs only j0-15  (input partitions   0:128)
#   A1: x in [33, 64) -> needs only j16-31 (input partitions 128:256)
#   mid: x in {31, 32} -> j15 (parts 96:128 of chunk0, rows zero-padded)
#                        + j16 (parts 0:32 of chunk1, rows zero-padded)
XA0 = list(range(0, 31))
XA1 = list(range(33, 64))
XMID = [31, 32]


def _mat_names(tag, mats):
    return {k: f"{tag}_{k[0]}_{'m' if k[1] < 0 else 'p'}{abs(k[1])}" for k in mats}


def build_nc(r2_keys, r3_keys, r4_keys):
    nc = bacc.Bacc('TRN2', target_bir_lowering=False, debug=False,
                   num_devices=N_CORES)

    lat = nc.declare_dram_parameter("latent_t", [4, BL], F32R, isOutput=False)
    w1 = nc.declare_dram_parameter("w1", [4, 256], F32R, isOutput=False)
    blin = nc.declare_dram_parameter("bl", [128, 2], F32, isOutput=False)
    r2n = _mat_names("r2", r2_keys)
    r3n = _mat_names("r3", r3_keys)
    r4n = _mat_names("r4", r4_keys)
    rd = {}
    for k, nm in list(r2n.items()) + list(r3n.items()):
        rd[nm] = nc.declare_dram_parameter(nm, [128, 2, 256], F16, isOutput=False)
    for k, nm in r4n.items():
        # support-split pieces of the [256, 384] fused L4 matrix
        rd[nm + "_a0"] = nc.declare_dram_parameter(
            nm + "_a0", [128, 6, len(XA0)], F16, isOutput=False)
        rd[nm + "_a1"] = nc.declare_dram_parameter(
            nm + "_a1", [128, 6, len(XA1)], F16, isOutput=False)
        rd[nm + "_m0"] = nc.declare_dram_parameter(
            nm + "_m0", [128, 6, 2], F16, isOutput=False)
        rd[nm + "_m1"] = nc.declare_dram_parameter(
            nm + "_m1", [128, 6, 2], F16, isOutput=False)
    # out stored (b, y, o, x) fp16; host transposes to NCHW + upcasts
    out = nc.declare_dram_parameter("out", [BL, 50, 6, 64], F16, isOutput=True)

    LR = mybir.ActivationFunctionType.Lrelu
    RELU = mybir.ActivationFunctionType.Relu

    with tile.TileContext(nc) as tc:
        with (
            tc.tile_pool(name="wpool", bufs=1) as wp,
            tc.tile_pool(name="acts", bufs=1) as ap,
            tc.tile_pool(name="ps", bufs=4, space=bass.MemorySpace.PSUM) as pp,
            tc.tile_pool(name="tmp", bufs=2) as tp,
            tc.tile_pool(name="outp", bufs=3) as op,
        ):
            lat_t = wp.tile([4, BL], F32R, tag="lat")
            nc.sync.dma_start(out=lat_t[:], in_=lat[:])
            w1_t = wp.tile([4, 256], F32R, tag="w1")
            nc.sync.dma_start(out=w1_t[:], in_=w1[:])
            bl_t = wp.tile([128, 2], F32, tag="bl")
            nc.sync.dma_start(out=bl_t[:], in_=blin[:])

            def load(nm, shape, dt):
                t = wp.tile(shape, dt, tag=nm)
                nc.sync.dma_start(out=t[:], in_=rd[nm][:])
                return t

            r2_t = {k: load(nm, [128, 2, 256], F16) for k, nm in r2n.items()}
            r3_t = {k: load(nm, [128, 2, 256], F16) for k, nm in r3n.items()}
            r4_t = {}
            for k, nm in r4n.items():
                r4_t[k] = (
                    load(nm + "_a0", [128, 6, len(XA0)], F16),
                    load(nm + "_a1", [128, 6, len(XA1)], F16),
                    load(nm + "_m0", [128, 6, 2], F16),
                    load(nm + "_m1", [128, 6, 2], F16),
                )

            # ---- L1: h[256, B] = leaky(W_lin.T @ lat + b)
            x1 = ap.tile([128, 2, BL], F16, tag="x1")
            ps1 = pp.tile([128, 2, 8, 64], F32, tag="ps")
            for mc in range(2):
                nc.tensor.matmul(
                    ps1[:, mc, :, :], w1_t[:, bass.ts(mc, 128)], lat_t[:],
                    start=True, stop=True,
                )
                nc.scalar.activation(
                    x1[:, mc, :], ps1[:, mc, :, :], LR,
                    bias=bl_t[:, mc:mc + 1], alpha=0.01,
                )

            # ---- L2: 256 -> 1280 (5 rows x 256).  x2 slot = 2*y + mc.
            # Per-row PSUM tile + per-row Act leaky (latency-critical: L3
            # consumes x2 almost immediately).
            x2 = ap.tile([128, 10, BL], F16, tag="x2")
            for y in range(5):
                ps = pp.tile([128, 2, 8, 64], F32, tag="ps")
                cs = _contribs(y, 0, 1, r2_t, 5)
                for mc in range(2):
                    n, tot = 0, len(cs) * 2
                    for (src, mt) in cs:
                        for kc in range(2):
                            nc.tensor.matmul(
                                ps[:, mc, :, :],
                                mt[:, kc, bass.ts(mc, 128)],
                                x1[:, kc, :],
                                start=(n == 0), stop=(n == tot - 1),
                            )
                            n += 1
                nc.scalar.activation(
                    x2[:, 2 * y:2 * y + 2, :], ps[:, :, :, :], LR, alpha=0.01,
                )

            # ---- L3: 1280 -> 6400 (25 rows x 256), fp16 out for L4.
            # x3 slot = 2*y + mc.  Per-row tiles; leaky on Act for 2/3 of
            # rows, DVE (2-instr mul+max) for every 3rd: keeps Act under the
            # PE rate.  DVE's extra latency is fine (L4 consumes much later).
            x3 = ap.tile([128, 50, BL], F16, tag="x3")
            for y in range(25):
                ps = pp.tile([128, 2, 8, 64], F32, tag="ps")
                i, p = divmod(y, 5)
                cs = _contribs(p, i, 5, r3_t, 5)
                for mc in range(2):
                    n, tot = 0, len(cs) * 2
                    for (src, mt) in cs:
                        for kc in range(2):
                            nc.tensor.matmul(
                                ps[:, mc, :, :],
                                mt[:, kc, bass.ts(mc, 128)],
                                x2[:, 2 * src + kc, :],
                                start=(n == 0), stop=(n == tot - 1),
                            )
                            n += 1
                if y % 3 != 2:
                    nc.scalar.activation(
                        x3[:, 2 * y:2 * y + 2, :], ps[:, :, :, :], LR,
                        alpha=0.01,
                    )
                else:
                    tmp = tp.tile([128, 2, 8, 64], F32, tag="tmp")
                    nc.vector.tensor_scalar_mul(tmp[:], ps[:], 0.01)
                    nc.vector.scalar_tensor_tensor(
                        x3[:, 2 * y:2 * y + 2, :], ps[:], 1.0, tmp[:],
                        op0=mybir.AluOpType.mult, op1=mybir.AluOpType.max,
                    )

            # ---- L4 (form 1, fp16): 6400 -> 19200, batch-major, relu, DMA.
            # 2-row PSUM tiles (bufs=4 ring -> no PE stalls), one relu act
            # per tile alternating Act/DVE, one DMA per 4-row ob group.
            actsel = 0
            for bb in range(BL // 128):
                for g0 in range(0, 50, 4):
                    ys = list(range(g0, min(g0 + 4, 50)))
                    ob = op.tile([128, 4, 6, 64], F16, tag="ob")
                    for h0 in range(0, len(ys), 2):
                        sub = ys[h0:h0 + 2]
                        pc = pp.tile([128, 2, 8, 64], F32, tag="ps")
                        for yy, y in enumerate(sub):
                            i, p = divmod(y, 2)
                            cs = _contribs(p, i, 25, r4_t, 2)
                            nmm, tot = 0, len(cs) * 4
                            for (src, (wa0, wa1, wm0, wm1)) in cs:
                                la = x3[:, 2 * src, bass.ts(bb, 128)]
                                lb = x3[:, 2 * src + 1, bass.ts(bb, 128)]
                                nc.tensor.matmul(
                                    pc[:, yy, 0:6, 0:31], la, wa0[:],
                                    start=(nmm == 0), stop=(nmm == tot - 4),
                                )
                                nc.tensor.matmul(
                                    pc[:, yy, 0:6, 33:64], lb, wa1[:],
                                    start=False, stop=(nmm == tot - 4),
                                )
                                nc.tensor.matmul(
                                    pc[:, yy, 0:6, 31:33], la, wm0[:],
                                    start=False, stop=False,
                                )
                                nc.tensor.matmul(
                                    pc[:, yy, 0:6, 31:33], lb, wm1[:],
                                    start=False, stop=(nmm == tot - 4),
                                )
                                nmm += 4
                        nsub = len(sub)
                        if actsel % 2 == 0:
                            nc.scalar.activation(
                                ob[:, h0:h0 + nsub, :, :],
                                pc[:, 0:nsub, 0:6, :], RELU)
                        else:
                            nc.vector.tensor_scalar_max(
                                ob[:, h0:h0 + nsub, :, :],
                                pc[:, 0:nsub, 0:6, :], 0.0)
                        actsel += 1
                    nc.sync.dma_start(
                        out=out[bass.ts(bb, 128), g0:g0 + len(ys), :, :],
                        in_=ob[:, 0:len(ys), :, :],
                    )
    nc.compile()
    return nc


# ---------------------------------------------------------------- entry
def kernel(**inputs):
    latent = np.asarray(inputs["latent_vector"], np.float32)
    W_lin, r2, r3, r4 = build_host_matrices(
        inputs["W_lin"], inputs["W_up1"], inputs["W_c1"],
        inputs["W_up2"], inputs["W_c2"], inputs["W_up3"], inputs["W_c3"],
    )
    b_lin = np.asarray(inputs["b_lin"], np.float32)

    if "nc" not in _CACHED:
        _CACHED["nc"] = build_nc(r2, r3, r4)
    nc = _CACHED["nc"]

    def pack(m):  # [256, N] -> [128, 2, N]
        return np.ascontiguousarray(
            m.reshape(2, 128, m.shape[1]).transpose(1, 0, 2))

    base = {"w1": np.ascontiguousarray(W_lin),
            "bl": np.ascontiguousarray(b_lin.reshape(2, 128).T)}
    for tag, mats in (("r2", r2), ("r3", r3)):
        for k, nm in _mat_names(tag, mats).items():
            base[nm] = pack(mats[k]).astype(np.float16)
    # L4 support-split weight pieces; cols of the [256, 384] matrix are
    # o*64 + x.  a0: rows j0-15 for x in XA0; a1: rows j16-31 for x in XA1;
    # m0/m1: rows j12-15 / j16-19 (only j15/j16 nonzero) for x in XMID.
    for k, nm in _mat_names("r4", r4).items():
        M = r4[k]  # [256, 384]
        cview = M.reshape(256, 6, 64)
        base[nm + "_a0"] = np.ascontiguousarray(
            cview[0:128][:, :, XA0].astype(np.float16))
        base[nm + "_a1"] = np.ascontiguousarray(
            cview[128:256][:, :, XA1].astype(np.float16))
        base[nm + "_m0"] = np.ascontiguousarray(
            cview[0:128][:, :, XMID].astype(np.float16))
        base[nm + "_m1"] = np.ascontiguousarray(
            cview[128:256][:, :, XMID].astype(np.float16))

    in_maps = []
    for c in range(N_CORES):
        sh = latent[c * BL:(c + 1) * BL]
        in_maps.append({**base,
                        "latent_t": np.ascontiguousarray(sh.T)})

    _CACHED["maps"] = in_maps
    res = run_bass_kernel_spmd(nc, in_maps, list(range(N_CORES)))
    outs = [np.transpose(r["out"].astype(np.float32), (0, 2, 1, 3))
            for r in res.results]
    return np.ascontiguousarray(np.concatenate(outs, axis=0))


if __name__ == "__main__":
    rng = np.random.default_rng(0)
    fake = {
        "latent_vector": rng.standard_normal((B, 4)).astype(np.float32),
        "W_lin": rng.standard_normal((4, 256)).astype(np.float32) * 0.5,
        "b_lin": np.zeros(256, np.float32),
        "W_up1": rng.standard_normal((5, 2, 32, 32)).astype(np.float32) * 0.1,
        "W_c1": rng.standard_normal((3, 3, 32, 16)).astype(np.float32) * 0.1,
        "W_up2": rng.standard_normal((5, 2, 16, 16)).astype(np.float32) * 0.1,
        "W_c2": rng.standard_normal((3, 3, 16, 8)).astype(np.float32) * 0.1,
        "W_up3": rng.standard_normal((2, 2, 8, 8)).astype(np.float32) * 0.1,
        "W_c3": rng.standard_normal((3, 3, 8, 6)).astype(np.float32) * 0.1,
    }
    o = kernel(**fake)
    print("kernel out", o.shape, o.dtype)


# revision 11
# speedup vs baseline: 1.4559x; 1.1043x over previous
"""Trainium2 Bass kernel for nn_BetaVAEMark10Decoder.

Network (per sample): latent(4) -> Linear(256)+leaky -> reshape (1,8,32)
 -> convT(5,2)s(5,2) -> conv3x3 SAME +leaky   (5,16,16)
 -> convT(5,2)s(5,2) -> conv3x3 SAME +leaky   (25,32,8)
 -> convT(2,2)s(2,2) -> conv3x3 SAME +relu    (50,64,6)  -> NCHW out.

Each convT(k=s) + 3x3 pair composes into one exact linear map, block-banded
over rows: out row y reads 1-2 input rows through per-phase matrices.

Cost model facts this kernel is built around:
  * matmul cost = out_free_size x cycles_per_row (independent of K);
    f32r is 1 cyc/row only when N >= 256; fp16/bf16 are 1 cyc/row always.
  * DMA transfers serialize at ~360 GB/s -> write the output as fp16.
  * Activation cost = free_size x 0.83ns + ~0.4us fixed -> merge acts
    across PSUM banks.

Layers:
  L1-L3 run form 0 (feature-major, N=512 batch free) in f32r.
  L4 runs form 1 (batch-major out) in fp16 with support-split x-groups:
    out cols x in [0,31) only need input cols j0-15 (one 128-part chunk),
    x in [33,64) only need j16-31, and x in {31,32} get two tiny K=32
    matmuls (N=12) reading aligned partition slices with zero-padded
    weight rows.  This halves L4 PE time vs. the 2-pass K-chunk split.

Sharding: pure data parallel, batch 4096 -> 8 cores x 512.
"""

import sys

import numpy as np

sys.path.insert(0, "/opt/trn_rl_repo")

import concourse.bass as bass  # noqa: E402
import concourse.bacc as bacc  # noqa: E402
import concourse.mybir as mybir  # noqa: E402
from concourse import tile  # noqa: E402
from concourse.bass_utils import run_bass_kernel_spmd  # noqa: E402

N_CORES = 8
B = 4096
BL = B // N_CORES  # 512 per core
F32 = mybir.dt.float32
F32R = mybir.dt.float32r
F16 = mybir.dt.float16


# ---------------------------------------------------------------- host math
def _fused_matrices(Wup, Wc, sy, sx, Win, in_idx, out_idx, n_out_cols):
    """Compose convT(k=s=(sy,sx)) with 3x3 SAME conv into per-phase row
    matrices.  Returns {(p, delta): M} where out row y (p = y%sy, i = y//sy)
    accumulates  in_row[i+delta] @ M[(p, delta)]  over available deltas.
    x-edge clipping is baked into M; y-edge clipping == skipping absent rows.
    """
    Wup = np.asarray(Wup, np.float32)
    Wc = np.asarray(Wc, np.float32)
    Cin = Wup.shape[2]
    Wout = Win * sx
    mats = {}
    for p in range(sy):
        deltas = {0}
        if p == 0:
            deltas.add(-1)
        if p == sy - 1:
            deltas.add(1)
        for d in sorted(deltas):
            M = np.zeros((Win * Cin, n_out_cols), np.float32)
            y = sy + p  # representative interior row
            i_t = y // sy + d
            nz = False
            for dy in (-1, 0, 1):
                yp = y + dy
                if yp // sy != i_t:
                    continue
                py = yp % sy
                for x in range(Wout):
                    for dx in (-1, 0, 1):
                        xp = x + dx
                        if xp < 0 or xp >= Wout:
                            continue
                        j, qx = divmod(xp, sx)
                        # conv_transpose (transpose_kernel=False) applies the
                        # spatially mirrored kernel per phase
                        CC = Wup[sy - 1 - py, sx - 1 - qx] @ Wc[dy + 1, dx + 1]
                        M[np.ix_(in_idx(j), out_idx(x))] += CC
                        nz = True
            if nz:
                mats[(p, d)] = M
    return mats


def build_host_matrices(W_lin, W_up1, W_c1, W_up2, W_c2, W_up3, W_c3):
    # L2 input = h natural ordering: feat = c*8 + j   (c<32, j<8)
    r2 = _fused_matrices(
        W_up1, W_c1, 5, 2, 8,
        in_idx=lambda j: np.arange(32) * 8 + j,
        out_idx=lambda x: x * 16 + np.arange(16),
        n_out_cols=256,
    )
    # L3 input ordering: feat = j*16 + c ; output feat = x*8 + o
    r3 = _fused_matrices(
        W_up2, W_c2, 5, 2, 16,
        in_idx=lambda j: j * 16 + np.arange(16),
        out_idx=lambda x: x * 8 + np.arange(8),
        n_out_cols=256,
    )
    # L4 input ordering: feat = j*8 + c ; output col = o*64 + x  (x contig)
    r4 = _fused_matrices(
        W_up3, W_c3, 2, 2, 32,
        in_idx=lambda j: j * 8 + np.arange(8),
        out_idx=lambda x: x + 64 * np.arange(6),
        n_out_cols=384,
    )
    return np.asarray(W_lin, np.float32), r2, r3, r4


def _contribs(p, i, n_in_rows, mats, sy):
    out = []
    for d in (-1, 0, 1):
        if (p, d) in mats and 0 <= i + d < n_in_rows:
            out.append((i + d, mats[(p, d)]))
    return out


def numpy_forward(latent, W_lin, b_lin, r2, r3, r4):
    """Pure-numpy forward through the fused matrices (golden check)."""
    def leaky(x):
        return np.where(x > 0, x, 0.01 * x)

    h = leaky(latent.astype(np.float32) @ W_lin + b_lin)  # [B, 256]
    rows = h[:, None, :]  # [B, 1, 256]
    for (mats, sy, n_in) in ((r2, 5, 1), (r3, 5, 5)):
        nrows = n_in * sy
        out = np.zeros((h.shape[0], nrows, 256), np.float32)
        for y in range(nrows):
            i, p = divmod(y, sy)
            for (src, M) in _contribs(p, i, n_in, mats, sy):
                out[:, y] += rows[:, src] @ M
        rows = leaky(out)
    out = np.zeros((h.shape[0], 50, 384), np.float32)
    for y in range(50):
        i, p = divmod(y, 2)
        for (src, M) in _contribs(p, i, 25, r4, 2):
            out[:, y] += rows[:, src] @ M
    out = np.maximum(out, 0.0)
    # [B, 50, 6, 64] -> NCHW [B, 6, 50, 64]
    return out.reshape(-1, 50, 6, 64).transpose(0, 2, 1, 3)


# ---------------------------------------------------------------- bass build
_CACHED = {}

# L4 support-split column groups (out col = o*64 + x):
#   A0: x in [0, 31)  -> needs only j0-15  (input partitions   0:128)
#   A1: x in [33, 64) -> needs only j16-31 (input partitions 128:256)
#   mid: x in {31, 32} -> j15 (parts 96:128 of chunk0, rows zero-padded)
#                        + j16 (parts 0:32 of chunk1, rows zero-padded)
XA0 = list(range(0, 31))
XA1 = list(range(33, 64))
XMID = [31, 32]


def _mat_names(tag, mats):
    return {k: f"{tag}_{k[0]}_{'m' if k[1] < 0 else 'p'}{abs(k[1])}" for k in mats}


def build_nc(r2_keys, r3_keys, r4_keys):
    nc = bacc.Bacc('TRN2', target_bir_lowering=False, debug=False,
                   num_devices=N_CORES)

    lat = nc.declare_dram_parameter("latent_t", [4, BL], F32R, isOutput=False)
    w1 = nc.declare_dram_parameter("w1", [4, 256], F32R, isOutput=False)
    blin = nc.declare_dram_parameter("bl", [128, 2], F32, isOutput=False)
    r2n = _mat_names("r2", r2_keys)
    r3n = _mat_names("r3", r3_keys)
    r4n = _mat_names("r4", r4_keys)
    rd = {}
    for k, nm in list(r2n.items()) + list(r3n.items()):
        rd[nm] = nc.declare_dram_parameter(nm, [128, 2, 256], F16, isOutput=False)
    for k, nm in r4n.items():
        # support-split pieces of the [256, 384] fused L4 matrix
        rd[nm + "_a0"] = nc.declare_dram_parameter(
            nm + "_a0", [128, 6, len(XA0)], F16, isOutput=False)
        rd[nm + "_a1"] = nc.declare_dram_parameter(
            nm + "_a1", [128, 6, len(XA1)], F16, isOutput=False)
        rd[nm + "_m0"] = nc.declare_dram_parameter(
            nm + "_m0", [128, 6, 2], F16, isOutput=False)
        rd[nm + "_m1"] = nc.declare_dram_parameter(
            nm + "_m1", [128, 6, 2], F16, isOutput=False)
    # out stored (b, y, o, x) fp16; host transposes to NCHW + upcasts
    out = nc.declare_dram_parameter("out", [BL, 50, 6, 64], F16, isOutput=True)

    LR = mybir.ActivationFunctionType.Lrelu
    RELU = mybir.ActivationFunctionType.Relu

    with tile.TileContext(nc) as tc:
        with (
            tc.tile_pool(name="wpool", bufs=1) as wp,
            tc.tile_pool(name="acts", bufs=1) as ap,
            tc.tile_pool(name="ps", bufs=4, space=bass.MemorySpace.PSUM) as pp,
            tc.tile_pool(name="tmp", bufs=2) as tp,
            tc.tile_pool(name="outp", bufs=6) as op,
        ):
            bl_t = wp.tile([128, 2], F32, tag="bl")
            nc.sync.dma_start(out=bl_t[:], in_=blin[:])
            lat_t = wp.tile([4, BL], F32R, tag="lat")
            nc.sync.dma_start(out=lat_t[:], in_=lat[:])
            w1_t = wp.tile([4, 256], F32R, tag="w1")
            nc.sync.dma_start(out=w1_t[:], in_=w1[:])

            def load(nm, shape, dt):
                t = wp.tile(shape, dt, tag=nm)
                nc.sync.dma_start(out=t[:], in_=rd[nm][:])
                return t

            r2_t = {k: load(nm, [128, 2, 256], F16) for k, nm in r2n.items()}
            r3_t = {k: load(nm, [128, 2, 256], F16) for k, nm in r3n.items()}
            r4_t = {}
            for k, nm in r4n.items():
                r4_t[k] = (
                    load(nm + "_a0", [128, 6, len(XA0)], F16),
                    load(nm + "_a1", [128, 6, len(XA1)], F16),
                    load(nm + "_m0", [128, 6, 2], F16),
                    load(nm + "_m1", [128, 6, 2], F16),
                )

            # ---- L1: h[256, B] = leaky(W_lin.T @ lat + b)
            x1 = ap.tile([128, 2, BL], F16, tag="x1")
            ps1 = pp.tile([128, 2, 8, 64], F32, tag="ps")
            for mc in range(2):
                nc.tensor.matmul(
                    ps1[:, mc, :, :], w1_t[:, bass.ts(mc, 128)], lat_t[:],
                    start=True, stop=True,
                )
                nc.scalar.activation(
                    x1[:, mc, :], ps1[:, mc, :, :], LR,
                    bias=bl_t[:, mc:mc + 1], alpha=0.01,
                )

            # ---- L2: 256 -> 1280 (5 rows x 256).  x2 slot = 2*y + mc.
            # Per-row PSUM tile + per-row Act leaky (latency-critical: L3
            # consumes x2 almost immediately).
            x2 = ap.tile([128, 10, BL], F16, tag="x2")
            for y in range(5):
                ps = pp.tile([128, 2, 8, 64], F32, tag="ps")
                cs = _contribs(y, 0, 1, r2_t, 5)
                for mc in range(2):
                    n, tot = 0, len(cs) * 2
                    for (src, mt) in cs:
                        for kc in range(2):
                            nc.tensor.matmul(
                                ps[:, mc, :, :],
                                mt[:, kc, bass.ts(mc, 128)],
                                x1[:, kc, :],
                                start=(n == 0), stop=(n == tot - 1),
                            )
                            n += 1
                nc.scalar.activation(
                    x2[:, 2 * y:2 * y + 2, :], ps[:, :, :, :], LR, alpha=0.01,
                )

            # ---- L3: 1280 -> 6400 (25 rows x 256), fp16 out for L4.
            # x3 slot = 2*y + mc.  Per-row tiles; leaky on Act for 2/3 of
            # rows, DVE (2-instr mul+max) for every 3rd: keeps Act under the
            # PE rate.  DVE's extra latency is fine (L4 consumes much later).
            x3 = ap.tile([128, 50, BL], F16, tag="x3")
            for y in range(25):
                ps = pp.tile([128, 2, 8, 64], F32, tag="ps")
                i, p = divmod(y, 5)
                cs = _contribs(p, i, 5, r3_t, 5)
                for mc in range(2):
                    n, tot = 0, len(cs) * 2
                    for (src, mt) in cs:
                        for kc in range(2):
                            nc.tensor.matmul(
                                ps[:, mc, :, :],
                                mt[:, kc, bass.ts(mc, 128)],
                                x2[:, 2 * src + kc, :],
                                start=(n == 0), stop=(n == tot - 1),
                            )
                            n += 1
                if y % 3 != 2:
                    nc.scalar.activation(
                        x3[:, 2 * y:2 * y + 2, :], ps[:, :, :, :], LR,
                        alpha=0.01,
                    )
                else:
                    tmp = tp.tile([128, 2, 8, 64], F32, tag="tmp")
                    nc.vector.tensor_scalar_mul(tmp[:], ps[:], 0.01)
                    nc.vector.scalar_tensor_tensor(
                        x3[:, 2 * y:2 * y + 2, :], ps[:], 1.0, tmp[:],
                        op0=mybir.AluOpType.mult, op1=mybir.AluOpType.max,
                    )

            # ---- L4 (form 1, fp16): 6400 -> 19200, batch-major, relu, DMA.
            # 2-row PSUM tiles (bufs=4 ring -> no PE stalls), one relu act
            # per tile alternating Act/DVE, one DMA per 4-row ob group.
            actsel = 0
            for bb in range(BL // 128):
                for g0 in range(0, 50, 4):
                    ys = list(range(g0, min(g0 + 4, 50)))
                    ob = op.tile([128, 4, 6, 64], F16, tag="ob")
                    for h0 in range(0, len(ys), 2):
                        sub = ys[h0:h0 + 2]
                        pc = pp.tile([128, 2, 8, 64], F32, tag="ps")
                        for yy, y in enumerate(sub):
                            i, p = divmod(y, 2)
                            cs = _contribs(p, i, 25, r4_t, 2)
                            nmm, tot = 0, len(cs) * 4
                            for (src, (wa0, wa1, wm0, wm1)) in cs:
                                la = x3[:, 2 * src, bass.ts(bb, 128)]
                                lb = x3[:, 2 * src + 1, bass.ts(bb, 128)]
                                nc.tensor.matmul(
                                    pc[:, yy, 0:6, 0:31], la, wa0[:],
                                    start=(nmm == 0), stop=(nmm == tot - 4),
                                )
                                nc.tensor.matmul(
                                    pc[:, yy, 0:6, 33:64], lb, wa1[:],
                                    start=False, stop=(nmm == tot - 4),
                                )
                                nc.tensor.matmul(
                                    pc[:, yy, 0:6, 31:33], la, wm0[:],
                                    start=False, stop=False,
                                )
                                nc.tensor.matmul(
                                    pc[:, yy, 0:6, 31:33], lb, wm1[:],
                                    start=False, stop=(nmm == tot - 4),
                                )
                                nmm += 4
                        nsub = len(sub)
                        if actsel % 2 == 0:
                            nc.scalar.activation(
                                ob[:, h0:h0 + nsub, :, :],
                                pc[:, 0:nsub, 0:6, :], RELU)
                        else:
                            nc.vector.tensor_scalar_max(
                                ob[:, h0:h0 + nsub, :, :],
                                pc[:, 0:nsub, 0:6, :], 0.0)
                        actsel += 1
                    nc.sync.dma_start(
                        out=out[bass.ts(bb, 128), g0:g0 + len(ys), :, :],
                        in_=ob[:, 0:len(ys), :, :],
                    )
    nc.compile()
    return nc


# ---------------------------------------------------------------- entry
def kernel(**inputs):
    latent = np.asarray(inputs["latent_vector"], np.float32)
    W_lin, r2, r3, r4 = build_host_matrices(
        inputs["W_lin"], inputs["W_up1"], inputs["W_c1"],
        inputs["W_up2"], inputs["W_c2"], inputs["W_up3"], inputs["W_c3"],
    )
    b_lin = np.asarray(inputs["b_lin"], np.float32)

    if "nc" not in _CACHED:
        _CACHED["nc"] = build_nc(r2, r3, r4)
    nc = _CACHED["nc"]

    def pack(m):  # [256, N] -> [128, 2, N]
        return np.ascontiguousarray(
            m.reshape(2, 128, m.shape[1]).transpose(1, 0, 2))

    base = {"w1": np.ascontiguousarray(W_lin),
            "bl": np.ascontiguousarray(b_lin.reshape(2, 128).T)}
    for tag, mats in (("r2", r2), ("r3", r3)):
        for k, nm in _mat_names(tag, mats).items():
            base[nm] = pack(mats[k]).astype(np.float16)
    # L4 support-split weight pieces; cols of the [256, 384] matrix are
    # o*64 + x.  a0: rows j0-15 for x in XA0; a1: rows j16-31 for x in XA1;
    # m0/m1: rows j12-15 / j16-19 (only j15/j16 nonzero) for x in XMID.
    for k, nm in _mat_names("r4", r4).items():
        M = r4[k]  # [256, 384]
        cview = M.reshape(256, 6, 64)
        base[nm + "_a0"] = np.ascontiguousarray(
            cview[0:128][:, :, XA0].astype(np.float16))
        base[nm + "_a1"] = np.ascontiguousarray(
            cview[128:256][:, :, XA1].astype(np.float16))
        base[nm + "_m0"] = np.ascontiguousarray(
            cview[0:128][:, :, XMID].astype(np.float16))
        base[nm + "_m1"] = np.ascontiguousarray(
            cview[128:256][:, :, XMID].astype(np.float16))

    in_maps = []
    for c in range(N_CORES):
        sh = latent[c * BL:(c + 1) * BL]
        in_maps.append({**base,
                        "latent_t": np.ascontiguousarray(sh.T)})

    _CACHED["maps"] = in_maps
    res = run_bass_kernel_spmd(nc, in_maps, list(range(N_CORES)))
    outs = [np.transpose(r["out"].astype(np.float32), (0, 2, 1, 3))
            for r in res.results]
    return np.ascontiguousarray(np.concatenate(outs, axis=0))


if __name__ == "__main__":
    rng = np.random.default_rng(0)
    fake = {
        "latent_vector": rng.standard_normal((B, 4)).astype(np.float32),
        "W_lin": rng.standard_normal((4, 256)).astype(np.float32) * 0.5,
        "b_lin": np.zeros(256, np.float32),
        "W_up1": rng.standard_normal((5, 2, 32, 32)).astype(np.float32) * 0.1,
        "W_c1": rng.standard_normal((3, 3, 32, 16)).astype(np.float32) * 0.1,
        "W_up2": rng.standard_normal((5, 2, 16, 16)).astype(np.float32) * 0.1,
        "W_c2": rng.standard_normal((3, 3, 16, 8)).astype(np.float32) * 0.1,
        "W_up3": rng.standard_normal((2, 2, 8, 8)).astype(np.float32) * 0.1,
        "W_c3": rng.standard_normal((3, 3, 8, 6)).astype(np.float32) * 0.1,
    }
    o = kernel(**fake)
    print("kernel out", o.shape, o.dtype)


# revision 12
# speedup vs baseline: 1.4989x; 1.0295x over previous
"""Trainium2 Bass kernel for nn_BetaVAEMark10Decoder.

Network (per sample): latent(4) -> Linear(256)+leaky -> reshape (1,8,32)
 -> convT(5,2)s(5,2) -> conv3x3 SAME +leaky   (5,16,16)
 -> convT(5,2)s(5,2) -> conv3x3 SAME +leaky   (25,32,8)
 -> convT(2,2)s(2,2) -> conv3x3 SAME +relu    (50,64,6)  -> NCHW out.

Each convT(k=s) + 3x3 pair composes into one exact linear map, block-banded
over rows: out row y reads 1-2 input rows through per-phase matrices.

Cost model facts this kernel is built around:
  * matmul cost = out_free_size x cycles_per_row (independent of K);
    f32r is 1 cyc/row only when N >= 256; fp16/bf16 are 1 cyc/row always.
  * DMA transfers serialize at ~360 GB/s -> write the output as fp16.
  * Activation cost = free_size x 0.83ns + ~0.4us fixed -> merge acts
    across PSUM banks.

Layers:
  L1-L3 run form 0 (feature-major, N=512 batch free) in f32r.
  L4 runs form 1 (batch-major out) in fp16 with support-split x-groups:
    out cols x in [0,31) only need input cols j0-15 (one 128-part chunk),
    x in [33,64) only need j16-31, and x in {31,32} get two tiny K=32
    matmuls (N=12) reading aligned partition slices with zero-padded
    weight rows.  This halves L4 PE time vs. the 2-pass K-chunk split.

Sharding: pure data parallel, batch 4096 -> 8 cores x 512.
"""

import sys

import numpy as np

sys.path.insert(0, "/opt/trn_rl_repo")

import concourse.bass as bass  # noqa: E402
import concourse.bacc as bacc  # noqa: E402
import concourse.mybir as mybir  # noqa: E402
from concourse import tile  # noqa: E402
from concourse.bass_utils import run_bass_kernel_spmd  # noqa: E402

N_CORES = 8
B = 4096
BL = B // N_CORES  # 512 per core
F32 = mybir.dt.float32
F32R = mybir.dt.float32r
F16 = mybir.dt.float16


# ---------------------------------------------------------------- host math
def _fused_matrices(Wup, Wc, sy, sx, Win, in_idx, out_idx, n_out_cols):
    """Compose convT(k=s=(sy,sx)) with 3x3 SAME conv into per-phase row
    matrices.  Returns {(p, delta): M} where out row y (p = y%sy, i = y//sy)
    accumulates  in_row[i+delta] @ M[(p, delta)]  over available deltas.
    x-edge clipping is baked into M; y-edge clipping == skipping absent rows.
    """
    Wup = np.asarray(Wup, np.float32)
    Wc = np.asarray(Wc, np.float32)
    Cin = Wup.shape[2]
    Wout = Win * sx
    mats = {}
    for p in range(sy):
        deltas = {0}
        if p == 0:
            deltas.add(-1)
        if p == sy - 1:
            deltas.add(1)
        for d in sorted(deltas):
            M = np.zeros((Win * Cin, n_out_cols), np.float32)
            y = sy + p  # representative interior row
            i_t = y // sy + d
            nz = False
            for dy in (-1, 0, 1):
                yp = y + dy
                if yp // sy != i_t:
                    continue
                py = yp % sy
                for x in range(Wout):
                    for dx in (-1, 0, 1):
                        xp = x + dx
                        if xp < 0 or xp >= Wout:
                            continue
                        j, qx = divmod(xp, sx)
                        # conv_transpose (transpose_kernel=False) applies the
                        # spatially mirrored kernel per phase
                        CC = Wup[sy - 1 - py, sx - 1 - qx] @ Wc[dy + 1, dx + 1]
                        M[np.ix_(in_idx(j), out_idx(x))] += CC
                        nz = True
            if nz:
                mats[(p, d)] = M
    return mats


def build_host_matrices(W_lin, W_up1, W_c1, W_up2, W_c2, W_up3, W_c3):
    # L2 input = h natural ordering: feat = c*8 + j   (c<32, j<8)
    r2 = _fused_matrices(
        W_up1, W_c1, 5, 2, 8,
        in_idx=lambda j: np.arange(32) * 8 + j,
        out_idx=lambda x: x * 16 + np.arange(16),
        n_out_cols=256,
    )
    # L3 input ordering: feat = j*16 + c ; output feat = x*8 + o
    r3 = _fused_matrices(
        W_up2, W_c2, 5, 2, 16,
        in_idx=lambda j: j * 16 + np.arange(16),
        out_idx=lambda x: x * 8 + np.arange(8),
        n_out_cols=256,
    )
    # L4 input ordering: feat = j*8 + c ; output col = o*64 + x  (x contig)
    r4 = _fused_matrices(
        W_up3, W_c3, 2, 2, 32,
        in_idx=lambda j: j * 8 + np.arange(8),
        out_idx=lambda x: x + 64 * np.arange(6),
        n_out_cols=384,
    )
    return np.asarray(W_lin, np.float32), r2, r3, r4


def _contribs(p, i, n_in_rows, mats, sy):
    out = []
    for d in (-1, 0, 1):
        if (p, d) in mats and 0 <= i + d < n_in_rows:
            out.append((i + d, mats[(p, d)]))
    return out


def numpy_forward(latent, W_lin, b_lin, r2, r3, r4):
    """Pure-numpy forward through the fused matrices (golden check)."""
    def leaky(x):
        return np.where(x > 0, x, 0.01 * x)

    h = leaky(latent.astype(np.float32) @ W_lin + b_lin)  # [B, 256]
    rows = h[:, None, :]  # [B, 1, 256]
    for (mats, sy, n_in) in ((r2, 5, 1), (r3, 5, 5)):
        nrows = n_in * sy
        out = np.zeros((h.shape[0], nrows, 256), np.float32)
        for y in range(nrows):
            i, p = divmod(y, sy)
            for (src, M) in _contribs(p, i, n_in, mats, sy):
                out[:, y] += rows[:, src] @ M
        rows = leaky(out)
    out = np.zeros((h.shape[0], 50, 384), np.float32)
    for y in range(50):
        i, p = divmod(y, 2)
        for (src, M) in _contribs(p, i, 25, r4, 2):
            out[:, y] += rows[:, src] @ M
    out = np.maximum(out, 0.0)
    # [B, 50, 6, 64] -> NCHW [B, 6, 50, 64]
    return out.reshape(-1, 50, 6, 64).transpose(0, 2, 1, 3)


# ---------------------------------------------------------------- bass build
_CACHED = {}

# L4 support-split column groups (out col = o*64 + x):
#   A0: x in [0, 31)  -> needs only j0-15  (input partitions   0:128)
#   A1: x in [33, 64) -> needs only j16-31 (input partitions 128:256)
#   mid: x in {31, 32} -> j15 (parts 96:128 of chunk0, rows zero-padded)
#                        + j16 (parts 0:32 of chunk1, rows zero-padded)
XA0 = list(range(0, 31))
XA1 = list(range(33, 64))
XMID = [31, 32]


def _mat_names(tag, mats):
    return {k: f"{tag}_{k[0]}_{'m' if k[1] < 0 else 'p'}{abs(k[1])}" for k in mats}


def build_nc(r2_keys, r3_keys, r4_keys, has_bias):
    nc = bacc.Bacc('TRN2', target_bir_lowering=False, debug=False,
                   num_devices=N_CORES)

    lw = nc.declare_dram_parameter("lw", [4, BL + 256], F32R, isOutput=False)
    blin = nc.declare_dram_parameter("bl", [128, 2], F32, isOutput=False)
    r2n = _mat_names("r2", r2_keys)
    r3n = _mat_names("r3", r3_keys)
    r4n = _mat_names("r4", r4_keys)
    rd = {}
    for k, nm in list(r2n.items()) + list(r3n.items()):
        rd[nm] = nc.declare_dram_parameter(nm, [128, 2, 256], F16, isOutput=False)
    for k, nm in r4n.items():
        # support-split pieces of the [256, 384] fused L4 matrix
        rd[nm + "_a0"] = nc.declare_dram_parameter(
            nm + "_a0", [128, 6, len(XA0)], F16, isOutput=False)
        rd[nm + "_a1"] = nc.declare_dram_parameter(
            nm + "_a1", [128, 6, len(XA1)], F16, isOutput=False)
        rd[nm + "_m0"] = nc.declare_dram_parameter(
            nm + "_m0", [128, 6, 2], F16, isOutput=False)
        rd[nm + "_m1"] = nc.declare_dram_parameter(
            nm + "_m1", [128, 6, 2], F16, isOutput=False)
    # out stored (b, y, o, x) fp16; host transposes to NCHW + upcasts
    out = nc.declare_dram_parameter("out", [BL, 50, 6, 64], F16, isOutput=True)

    LR = mybir.ActivationFunctionType.Lrelu
    RELU = mybir.ActivationFunctionType.Relu

    with tile.TileContext(nc) as tc:
        with (
            tc.tile_pool(name="wpool", bufs=1) as wp,
            tc.tile_pool(name="acts", bufs=1) as ap,
            tc.tile_pool(name="ps", bufs=4, space=bass.MemorySpace.PSUM) as pp,
            tc.tile_pool(name="tmp", bufs=2) as tp,
            tc.tile_pool(name="outp", bufs=6) as op,
        ):
            lw_t = wp.tile([4, BL + 256], F32R, tag="lw")
            nc.sync.dma_start(out=lw_t[:], in_=lw[:])
            lat_t = lw_t[:, 0:BL]
            w1_t = lw_t[:, BL:BL + 256]
            if has_bias:
                bl_t = wp.tile([128, 2], F32, tag="bl")
                nc.sync.dma_start(out=bl_t[:], in_=blin[:])

            def load(nm, shape, dt):
                t = wp.tile(shape, dt, tag=nm)
                nc.sync.dma_start(out=t[:], in_=rd[nm][:])
                return t

            r2_t = {k: load(nm, [128, 2, 256], F16) for k, nm in r2n.items()}
            r3_t = {k: load(nm, [128, 2, 256], F16) for k, nm in r3n.items()}
            r4_t = {}
            for k, nm in r4n.items():
                r4_t[k] = (
                    load(nm + "_a0", [128, 6, len(XA0)], F16),
                    load(nm + "_a1", [128, 6, len(XA1)], F16),
                    load(nm + "_m0", [128, 6, 2], F16),
                    load(nm + "_m1", [128, 6, 2], F16),
                )

            # ---- L1: h[256, B] = leaky(W_lin.T @ lat + b)
            x1 = ap.tile([128, 2, BL], F16, tag="x1")
            ps1 = pp.tile([128, 2, 8, 64], F32, tag="ps")
            for mc in range(2):
                nc.tensor.matmul(
                    ps1[:, mc, :, :],
                    lw_t[:, BL + mc * 128:BL + (mc + 1) * 128],
                    lw_t[:, 0:BL],
                    start=True, stop=True,
                )
                if has_bias:
                    nc.scalar.activation(
                        x1[:, mc, :], ps1[:, mc, :, :], LR,
                        bias=bl_t[:, mc:mc + 1], alpha=0.01,
                    )
                else:
                    nc.scalar.activation(
                        x1[:, mc, :], ps1[:, mc, :, :], LR, alpha=0.01,
                    )

            # ---- L2: 256 -> 1280 (5 rows x 256).  x2 slot = 2*y + mc.
            # Per-row PSUM tile + per-row Act leaky (latency-critical: L3
            # consumes x2 almost immediately).
            x2 = ap.tile([128, 10, BL], F16, tag="x2")
            for y in range(5):
                ps = pp.tile([128, 2, 8, 64], F32, tag="ps")
                cs = _contribs(y, 0, 1, r2_t, 5)
                for mc in range(2):
                    n, tot = 0, len(cs) * 2
                    for (src, mt) in cs:
                        for kc in range(2):
                            nc.tensor.matmul(
                                ps[:, mc, :, :],
                                mt[:, kc, bass.ts(mc, 128)],
                                x1[:, kc, :],
                                start=(n == 0), stop=(n == tot - 1),
                            )
                            n += 1
                nc.scalar.activation(
                    x2[:, 2 * y:2 * y + 2, :], ps[:, :, :, :], LR, alpha=0.01,
                )

            # ---- L3: 1280 -> 6400 (25 rows x 256), fp16 out for L4.
            # x3 slot = 2*y + mc.  Per-row tiles; leaky on Act for 2/3 of
            # rows, DVE (2-instr mul+max) for every 3rd: keeps Act under the
            # PE rate.  DVE's extra latency is fine (L4 consumes much later).
            x3 = ap.tile([128, 50, BL], F16, tag="x3")
            for y in range(25):
                ps = pp.tile([128, 2, 8, 64], F32, tag="ps")
                i, p = divmod(y, 5)
                cs = _contribs(p, i, 5, r3_t, 5)
                for mc in range(2):
                    n, tot = 0, len(cs) * 2
                    for (src, mt) in cs:
                        for kc in range(2):
                            nc.tensor.matmul(
                                ps[:, mc, :, :],
                                mt[:, kc, bass.ts(mc, 128)],
                                x2[:, 2 * src + kc, :],
                                start=(n == 0), stop=(n == tot - 1),
                            )
                            n += 1
                if y % 3 != 2:
                    nc.scalar.activation(
                        x3[:, 2 * y:2 * y + 2, :], ps[:, :, :, :], LR,
                        alpha=0.01,
                    )
                else:
                    tmp = tp.tile([128, 2, 8, 64], F32, tag="tmp")
                    nc.vector.tensor_scalar_mul(tmp[:], ps[:], 0.01)
                    nc.vector.scalar_tensor_tensor(
                        x3[:, 2 * y:2 * y + 2, :], ps[:], 1.0, tmp[:],
                        op0=mybir.AluOpType.mult, op1=mybir.AluOpType.max,
                    )

            # ---- L4 (form 1, fp16): 6400 -> 19200, batch-major, relu, DMA.
            # 2-row PSUM tiles (bufs=4 ring -> no PE stalls), one relu act
            # per tile alternating Act/DVE, one DMA per 4-row ob group.
            actsel = 0
            for bb in range(BL // 128):
                for g0 in range(0, 50, 4):
                    ys = list(range(g0, min(g0 + 4, 50)))
                    ob = op.tile([128, 4, 6, 64], F16, tag="ob")
                    for h0 in range(0, len(ys), 2):
                        sub = ys[h0:h0 + 2]
                        pc = pp.tile([128, 2, 8, 64], F32, tag="ps")
                        for yy, y in enumerate(sub):
                            i, p = divmod(y, 2)
                            cs = _contribs(p, i, 25, r4_t, 2)
                            nmm, tot = 0, len(cs) * 4
                            for (src, (wa0, wa1, wm0, wm1)) in cs:
                                la = x3[:, 2 * src, bass.ts(bb, 128)]
                                lb = x3[:, 2 * src + 1, bass.ts(bb, 128)]
                                nc.tensor.matmul(
                                    pc[:, yy, 0:6, 0:31], la, wa0[:],
                                    start=(nmm == 0), stop=(nmm == tot - 4),
                                )
                                nc.tensor.matmul(
                                    pc[:, yy, 0:6, 33:64], lb, wa1[:],
                                    start=False, stop=(nmm == tot - 4),
                                )
                                nc.tensor.matmul(
                                    pc[:, yy, 0:6, 31:33], la, wm0[:],
                                    start=False, stop=False,
                                )
                                nc.tensor.matmul(
                                    pc[:, yy, 0:6, 31:33], lb, wm1[:],
                                    start=False, stop=(nmm == tot - 4),
                                )
                                nmm += 4
                        nsub = len(sub)
                        if actsel % 2 == 0:
                            nc.scalar.activation(
                                ob[:, h0:h0 + nsub, :, :],
                                pc[:, 0:nsub, 0:6, :], RELU)
                        else:
                            nc.vector.tensor_scalar_max(
                                ob[:, h0:h0 + nsub, :, :],
                                pc[:, 0:nsub, 0:6, :], 0.0)
                        actsel += 1
                    nc.sync.dma_start(
                        out=out[bass.ts(bb, 128), g0:g0 + len(ys), :, :],
                        in_=ob[:, 0:len(ys), :, :],
                    )
    nc.compile()
    return nc


# ---------------------------------------------------------------- entry
def kernel(**inputs):
    latent = np.asarray(inputs["latent_vector"], np.float32)
    W_lin, r2, r3, r4 = build_host_matrices(
        inputs["W_lin"], inputs["W_up1"], inputs["W_c1"],
        inputs["W_up2"], inputs["W_c2"], inputs["W_up3"], inputs["W_c3"],
    )
    b_lin = np.asarray(inputs["b_lin"], np.float32)

    has_bias = bool(np.any(b_lin != 0.0))
    if "nc" not in _CACHED:
        _CACHED["nc"] = build_nc(r2, r3, r4, has_bias)
    nc = _CACHED["nc"]

    def pack(m):  # [256, N] -> [128, 2, N]
        return np.ascontiguousarray(
            m.reshape(2, 128, m.shape[1]).transpose(1, 0, 2))

    base = {"bl": np.ascontiguousarray(b_lin.reshape(2, 128).T)}
    for tag, mats in (("r2", r2), ("r3", r3)):
        for k, nm in _mat_names(tag, mats).items():
            base[nm] = pack(mats[k]).astype(np.float16)
    # L4 support-split weight pieces; cols of the [256, 384] matrix are
    # o*64 + x.  a0: rows j0-15 for x in XA0; a1: rows j16-31 for x in XA1;
    # m0/m1: rows j12-15 / j16-19 (only j15/j16 nonzero) for x in XMID.
    for k, nm in _mat_names("r4", r4).items():
        M = r4[k]  # [256, 384]
        cview = M.reshape(256, 6, 64)
        base[nm + "_a0"] = np.ascontiguousarray(
            cview[0:128][:, :, XA0].astype(np.float16))
        base[nm + "_a1"] = np.ascontiguousarray(
            cview[128:256][:, :, XA1].astype(np.float16))
        base[nm + "_m0"] = np.ascontiguousarray(
            cview[0:128][:, :, XMID].astype(np.float16))
        base[nm + "_m1"] = np.ascontiguousarray(
            cview[128:256][:, :, XMID].astype(np.float16))

    in_maps = []
    for c in range(N_CORES):
        sh = latent[c * BL:(c + 1) * BL]
        in_maps.append({**base,
                        "lw": np.ascontiguousarray(
                            np.concatenate([sh.T, W_lin], axis=1))})

    _CACHED["maps"] = in_maps
    res = run_bass_kernel_spmd(nc, in_maps, list(range(N_CORES)))
    outs = [np.transpose(r["out"].astype(np.float32), (0, 2, 1, 3))
            for r in res.results]
    return np.ascontiguousarray(np.concatenate(outs, axis=0))


if __name__ == "__main__":
    rng = np.random.default_rng(0)
    fake = {
        "latent_vector": rng.standard_normal((B, 4)).astype(np.float32),
        "W_lin": rng.standard_normal((4, 256)).astype(np.float32) * 0.5,
        "b_lin": np.zeros(256, np.float32),
        "W_up1": rng.standard_normal((5, 2, 32, 32)).astype(np.float32) * 0.1,
        "W_c1": rng.standard_normal((3, 3, 32, 16)).astype(np.float32) * 0.1,
        "W_up2": rng.standard_normal((5, 2, 16, 16)).astype(np.float32) * 0.1,
        "W_c2": rng.standard_normal((3, 3, 16, 8)).astype(np.float32) * 0.1,
        "W_up3": rng.standard_normal((2, 2, 8, 8)).astype(np.float32) * 0.1,
        "W_c3": rng.standard_normal((3, 3, 8, 6)).astype(np.float32) * 0.1,
    }
    o = kernel(**fake)
    print("kernel out", o.shape, o.dtype)
